# revision 17
# baseline (speedup 1.0000x reference)
"""Trainium2 Bass kernel for nn_BiClassifier (bilinear pairwise MLP).

Math (per batch b):
    in1 = input1 @ W1.T + b1            # [N1, HID]
    in2 = input2 @ W2.T                 # [N2, HID]
    h   = relu(in1[:,None,:] + in2[None,:,:])   # [N1, N2, HID]  (never materialized)
    out = h @ Wo.T + bo                 # [N1, N2, OUT]

Strategy: shard the 512 (b, n1) rows across 8 cores (64 rows each, one batch
per core pair). Weights are replicated. On each core:
  Phase A (PE): produce in1T [HID, 64] and in2T [HID, 128] with HID on the
      partition axis (8 blocks of 128), using host-pre-transposed weights/inputs.
  Phase B: h tiles [128, *] = relu(in2T_hp[:, m] + in1T_hp[:, n]) feed the PE,
      which contracts them against Wo embedded into [128, 16] stationary tiles
      (8 row-group slots x 2 outputs across 16 PSUM partitions); one PSUM bank
      [16, 512] accumulates 32 rows of output.
  h generation is split between two engines:
   - 'V' 16-row blocks: ONE custom DVE instruction (ADD_RELU_ANT, registered
     at import) computes relu(in2 bcast + in1 bcast) over FD=2048 via 3D
     access patterns -- 143 ns/row vs 244 ns/row for the stock dual-op
     tensor_scalar (which is overhead-bound at FD=128).
   - 'A' 4-row groups: ScalarE ACTIVATE Relu with per-partition bias reading
     in2 from PSUM (4 x FD=128, ~385 ns each).
Host pre/post: transpose/shard inputs, unscramble output, add bo.
"""

import sys

import numpy as np

_REPO = "/opt/trn_rl_repo"
if _REPO not in sys.path:
    sys.path.insert(0, _REPO)

import concourse.bass as bass
import concourse.mybir as mybir
import concourse.tile as tile
from concourse import bacc
from concourse.bass_utils import run_bass_kernel_spmd

B, N1, N2, D, HID, OUT = 4, 128, 128, 768, 1024, 2
NCORES = 8
NR = 64            # (b, n1) rows per core
DB = D // 128      # 6 contraction blocks for the input projections
HP = HID // 128    # 8 hid blocks
NSUP = 2           # row supers per core (32 rows each -> one PSUM bank)
NG = 8             # row groups per super
GR = 4             # rows per group (group -> one [128, 512] h tile)
BLK = 16           # rows per V-custom-instruction block (FD = BLK*N2 = 2048)

# Engine weights (V=custom-DVE 16-row blocks, A=ScalarE 16-row blocks) for
# the h add/relu work, scheduled over the 32 16-row blocks per core.
# Measured costs: V 2.20us/block, A 6.42us/block; A also carries the in2t
# copies + evac, so V:A = 24:8 balances the two streams.
ADD_W = (76, 24)
DT = "bfloat16"

_CACHE = {}


def _register_add_relu():
    """Idempotently register the fused ADD_RELU_ANT custom DVE op."""
    import concourse.dve_ops as dve_ops
    from concourse.dve_spec import Spec, Src0, Src1, relu, lower
    from concourse.dve_uop import DveOpSpec

    if "ADD_RELU_ANT" in dve_ops._SUB_OPCODE_FOR_NAME:
        return next(o for o in dve_ops.OPS if o.name == "ADD_RELU_ANT")

    spec = Spec(
        body=relu(Src0 + Src1),
        reference=lambda in0, in1, s0, s1, imm2: np.maximum(
            in0.astype(np.float32) + in1.astype(np.float32), 0.0
        ),
    )
    row = dve_ops._CUSTOM_DVE_ROW_BASE + len(dve_ops.OPS)
    assert row < 0x20
    shas = {}
    for ver in ("v3", "v4"):
        shas[ver] = DveOpSpec(
            name="ADD_RELU_ANT", opcode=row, uops=lower(spec, ver=ver), rd1_en=True
        ).sha(ver)
    op = dve_ops.DveOp("ADD_RELU_ANT", spec, subdim=False, uops_sha=shas)
    dve_ops.OPS.append(op)
    dve_ops.CUSTOM_DVE_SPECS[op.name] = spec
    dve_ops._SUB_OPCODE_FOR_NAME[op.name] = row
    return op


def _wrr(weights, n):
    """Weighted round-robin schedule of 'V'/'A' over n slots."""
    names = "VA"
    credits = [0.0] * len(weights)
    total = float(sum(weights))
    out = []
    for _ in range(n):
        credits = [c + w for c, w in zip(credits, weights)]
        i = max(range(len(weights)), key=lambda k: credits[k])
        credits[i] -= total
        out.append(names[i])
    return out


def _build(dt_name=None, add_w=None):
    f32 = mybir.dt.float32
    dt = getattr(mybir.dt, dt_name or DT)
    add_relu = _register_add_relu()
    n_blocks = NSUP * HP * (32 // BLK)   # 16-row blocks per core
    add_pat = _wrr(add_w or ADD_W, n_blocks)
    # Bacc (not plain Bass): its finalize() runs the walrus legalization
    # passes (move_matmul_waits_to_ldweights, event semaphores, ...) without
    # which multi-wait instructions fail neuronxcc codegen.
    nc = bacc.Bacc(None, target_bir_lowering=False)

    w1 = nc.declare_dram_parameter("w1", [128, HP * DB * 128], dt, isOutput=False)
    w2 = nc.declare_dram_parameter("w2", [128, HP * DB * 128], dt, isOutput=False)
    wote = nc.declare_dram_parameter("wote", [128, HP * NG * 16], dt, isOutput=False)
    b1s = nc.declare_dram_parameter("b1s", [128, HP], f32, isOutput=False)
    x1 = nc.declare_dram_parameter("x1", [128, DB * NR], dt, isOutput=False)
    x2 = nc.declare_dram_parameter("x2", [128, DB * N2], dt, isOutput=False)
    out = nc.declare_dram_parameter("out", [16, NSUP * 512], f32, isOutput=True)

    blk_idx = 0

    with tile.TileContext(nc) as tc:
        with (
            tc.tile_pool(name="const", bufs=1) as cpool,
            tc.tile_pool(name="wpool", bufs=1) as wpool,
            tc.tile_pool(name="hpool", bufs=6) as hpool,
            tc.tile_pool(name="hapool", bufs=24) as hapool,
            tc.tile_pool(name="pa", bufs=2, space=bass.MemorySpace.PSUM) as papool,
            tc.tile_pool(name="po", bufs=2, space=bass.MemorySpace.PSUM) as popool,
            tc.tile_pool(name="p2", bufs=1, space=bass.MemorySpace.PSUM) as p2pool,
        ):
            x1sb = cpool.tile([128, DB * NR], dt)
            x2sb = cpool.tile([128, DB * N2], dt)
            b1sb = cpool.tile([128, HP], f32)
            wotesb = cpool.tile([128, HP * NG * 16], dt)
            # in1t (fp32) feeds ACTIVATE bias per-partition reads (fp32-only);
            # in1tb (bf16) feeds the custom-DVE Src1 stream.
            in1t = cpool.tile([128, HP * NR], f32)
            in1tb = cpool.tile([128, HP * NR], dt, name="in1tb")
            in2t = cpool.tile([128, HP * N2], dt)
            outsb = cpool.tile([16, NSUP * 512], f32)

            # DMA order matters for pipeline fill: the first phase-A matmuls
            # need x1 + w1[0] (and x2 + w2[0]); defer b1/wote (phase-B-only).
            nc.sync.dma_start(out=x1sb[:], in_=x1[:])

            # Per-hp weight tiles so phase A hp can start as soon as its
            # slice lands (whole-W DMA would serialize ~20us at the front).
            w1sb = []
            w2sb = []
            for hp in range(HP):
                t1 = wpool.tile([128, DB * 128], dt, tag=f"w1_{hp}")
                t2 = wpool.tile([128, DB * 128], dt, tag=f"w2_{hp}")
                w1sb.append(t1)
                w2sb.append(t2)

            def _load_w(hp):
                # w2 first: the in2 path (proj -> ScalarE copy -> custom op)
                # is the longer dependency chain for each hp section.
                nc.sync.dma_start(
                    out=w2sb[hp][:], in_=w2[:, hp * DB * 128 : (hp + 1) * DB * 128]
                )
                nc.sync.dma_start(
                    out=w1sb[hp][:], in_=w1[:, hp * DB * 128 : (hp + 1) * DB * 128]
                )

            nc.sync.dma_start(out=x2sb[:], in_=x2[:])
            nc.sync.dma_start(out=b1sb[:], in_=b1s[:])
            _load_w(0)
            _load_w(1)
            nc.sync.dma_start(out=wotesb[:], in_=wote[:])
            for hp in range(2, HP):
                _load_w(hp)

            # Dummy activation up front: pulls the ~2.7us ACT table load into
            # the DMA fill window instead of the first real relu.
            warm = cpool.tile([128, 1], f32, name="warm")
            nc.vector.memset(warm[:], 0.0)
            nc.scalar.activation(
                warm[:], warm[:], mybir.ActivationFunctionType.Relu, bias=0.0,
                scale=1.0,
            )

            # in2 projections stay resident in PSUM (2 banks, 4 hid blocks
            # each): ScalarE reads PSUM faster than SBUF, so 'A' groups
            # consume these directly; 'V' blocks use the bf16 SBUF copy.
            in2ps = [
                p2pool.tile([128, 4 * N2], f32, name=f"in2ps{i}") for i in range(2)
            ]

            # V/A assignment over the 16-row blocks, in emission order
            # (hp-major) so the WRR spread matches the instruction stream.
            halves = {}
            for hp in range(HP):
                for sup in range(NSUP):
                    for half in range(32 // BLK):
                        halves[(sup, hp, half)] = add_pat[blk_idx]
                        blk_idx += 1

            pso = [popool.tile([16, 512], f32, name=f"pso{s}") for s in range(NSUP)]
            mm_idx = [0] * NSUP
            n_mm = NSUP * HP * 32 // GR // NSUP  # accumulating MMs per super

            def mm(sup, rhs, hp, g):
                nc.tensor.matmul(
                    pso[sup][:],
                    wotesb[:, hp * NG * 16 + g * 16 : hp * NG * 16 + (g + 1) * 16],
                    rhs,
                    start=(mm_idx[sup] == 0),
                    stop=(mm_idx[sup] == n_mm - 1),
                )
                mm_idx[sup] += 1

            def v_half(sup, hp, half):
                r0 = sup * 32 + half * BLK
                g0 = (half * BLK) // GR
                h = hpool.tile([128, BLK * N2], dt, tag="hv")
                hv = h[:].rearrange("p (j m) -> p j m", j=BLK)
                a = in2t[:, None, hp * N2 : (hp + 1) * N2].broadcast_to(
                    [128, BLK, N2]
                )
                bsl = in1tb[:, hp * NR + r0 : hp * NR + r0 + BLK]
                b = bsl[:, :, None].broadcast_to([128, BLK, N2])
                nc.vector._custom_dve(add_relu, out=hv, in0=a, in1=b)
                for k in range(BLK // GR):
                    mm(sup, h[:, k * GR * N2 : (k + 1) * GR * N2], hp, g0 + k)

            def a_half_gen(sup, hp, half):
                r0 = sup * 32 + half * BLK
                g0 = (half * BLK) // GR
                psrc = in2ps[hp // 4][:, (hp % 4) * N2 : (hp % 4 + 1) * N2]
                tiles = []
                for k in range(BLK // GR):
                    h = hapool.tile([128, GR * N2], dt, tag="ha")
                    for j in range(GR):
                        row = r0 + k * GR + j
                        col = in1t[:, hp * NR + row : hp * NR + row + 1]
                        nc.scalar.activation(
                            h[:, j * N2 : (j + 1) * N2],
                            psrc,
                            mybir.ActivationFunctionType.Relu,
                            bias=col,
                            scale=1.0,
                        )
                    tiles.append((sup, h, hp, g0 + k))
                return tiles

            # ---- Phases A+B fused per hid-block ----
            # The hp loop emits: projections for hp, then super-0's V halves
            # and A ACTIVATEs for hp. This gives every engine work as soon as
            # the first weight slices land, hiding the ~11us weight-DMA
            # stream. Each A half's 4 accumulating MMs are interleaved into
            # the global V-half MM stream at the point where ScalarE (paced
            # at ~6.4us per A half) will provably have produced its tiles, so
            # the PE (strict program order within a PSUM bank, and in
            # practice across the stream) never stalls on ScalarE output and
            # there is no pure-PE tail after the V stream ends.
            a_pending = []   # (a_idx, [(sup, tile, hp, g), ...])
            n_a = sum(1 for w in add_pat if w != "V")
            n_v = len(add_pat) - n_a
            v_count = 0
            a_count = 0
            sup0_mm_done = [False]

            def a_flush_point(j):
                # A half j is ready once ScalarE has run j+1 halves (~6.42us
                # each) == the time V takes for ~2.9 halves (2.2us each).
                return min(int((j + 1) * 2.8) + 1, n_v - 1)

            def evac(sup):
                # evac on ScalarE: VectorE is the critical stream.
                nc.scalar.copy(outsb[:, sup * 512 : (sup + 1) * 512], pso[sup][:])
                nc.sync.dma_start(
                    out=out[:, sup * 512 : (sup + 1) * 512],
                    in_=outsb[:, sup * 512 : (sup + 1) * 512],
                )

            def maybe_flush(force=False):
                nonlocal a_pending
                keep = []
                for j, tiles in a_pending:
                    if force or v_count >= a_flush_point(j):
                        for s, h, hp, g in tiles:
                            mm(s, h[:], hp, g)
                    else:
                        keep.append((j, tiles))
                a_pending = keep
                if not sup0_mm_done[0] and mm_idx[0] == n_mm:
                    sup0_mm_done[0] = True
                    evac(0)

            def emit_half(sup, hp, half):
                nonlocal v_count, a_count
                if halves[(sup, hp, half)] == "V":
                    v_half(sup, hp, half)
                    v_count += 1
                    maybe_flush()
                else:
                    a_pending.append((a_count, a_half_gen(sup, hp, half)))
                    a_count += 1

            for hp in range(HP):
                ps1 = papool.tile([128, NR], f32, tag="ps1")
                for db in range(DB):
                    nc.tensor.matmul(
                        ps1[:],
                        w1sb[hp][:, db * 128 : (db + 1) * 128],
                        x1sb[:, db * NR : (db + 1) * NR],
                        start=(db == 0),
                        stop=(db == DB - 1),
                    )
                # in1t must stay fp32: it feeds ACTIVATE bias scalar reads.
                # high_priority: these per-hp phase-A ops must jump ahead of
                # earlier-emitted phase-B work in the engine FIFOs (e.g. the
                # in2t copy would otherwise sit behind ~6.4us of A-half
                # ACTIVATEs from the previous hp section, stalling VectorE).
                with tc.high_priority():
                    nc.vector.tensor_scalar_add(
                        in1t[:, hp * NR : (hp + 1) * NR], ps1[:], b1sb[:, hp : hp + 1]
                    )
                    nc.vector.tensor_copy(
                        in1tb[:, hp * NR : (hp + 1) * NR],
                        in1t[:, hp * NR : (hp + 1) * NR],
                    )

                ps2 = in2ps[hp // 4][:, (hp % 4) * N2 : (hp % 4 + 1) * N2]
                for db in range(DB):
                    nc.tensor.matmul(
                        ps2,
                        w2sb[hp][:, db * 128 : (db + 1) * 128],
                        x2sb[:, db * N2 : (db + 1) * N2],
                        start=(db == 0),
                        stop=(db == DB - 1),
                    )
                with tc.high_priority():
                    nc.scalar.copy(in2t[:, hp * N2 : (hp + 1) * N2], ps2)

                # Both supers' halves for this hp: gives VectorE ~6.6us of
                # ready work per ~2.5us weight-DMA step, so the fill never
                # starves it after hp0. Both PSUM banks accumulate at once.
                for sup in range(NSUP):
                    for half in range(32 // BLK):
                        emit_half(sup, hp, half)

            maybe_flush(force=True)
            if not sup0_mm_done[0]:
                evac(0)
            evac(1)

    nc.finalize()
    return nc


def _np_dt(dt_name):
    if dt_name == "bfloat16":
        import ml_dtypes

        return ml_dtypes.bfloat16
    return np.float32


def _host_prep(input1, input2, W1, b1, W2, Wo, dt_name=None):
    f32 = np.float32
    dt = _np_dt(dt_name or DT)
    c = np.ascontiguousarray

    # w[p, hp, db, j] = W[hp*128+j, db*128+p]
    w1sb = c(W1.reshape(HP, 128, DB, 128).transpose(3, 0, 2, 1).reshape(128, -1), dt)
    w2sb = c(W2.reshape(HP, 128, DB, 128).transpose(3, 0, 2, 1).reshape(128, -1), dt)

    # wote[p, hp, s, 2s+o] = Wo[o, hp*128+p]
    wo_hpo = Wo.T.reshape(HP, 128, OUT)  # [hp, p, o]
    wote = np.zeros((128, HP, NG, 16), f32)
    for s in range(NG):
        wote[:, :, s, 2 * s : 2 * s + 2] = wo_hpo.transpose(1, 0, 2)
    wote = c(wote.reshape(128, -1), dt)

    b1sb = c(b1.reshape(HP, 128).T, f32)

    in_maps = []
    for core in range(NCORES):
        b, half = core // 2, core % 2
        n0 = half * NR
        x1sb = c(
            input1[b, n0 : n0 + NR].reshape(NR, DB, 128).transpose(2, 1, 0).reshape(128, -1),
            dt,
        )
        x2sb = c(
            input2[b].reshape(N2, DB, 128).transpose(2, 1, 0).reshape(128, -1), dt
        )
        in_maps.append(
            {"w1": w1sb, "w2": w2sb, "wote": wote, "b1s": b1sb, "x1": x1sb, "x2": x2sb}
        )
    return in_maps


def _host_post(results, bo):
    out_full = np.empty((B, N1, N2, OUT), np.float32)
    for core in range(NCORES):
        b, half = core // 2, core % 2
        co = np.asarray(results[core]["out"], np.float32)
        co = co.reshape(NG, OUT, NSUP, GR, N2)  # [s, o, sup, j, m]
        arr = co.transpose(2, 0, 3, 4, 1).reshape(NR, N2, OUT)  # [sup,s,j] -> rows
        out_full[b, half * NR : (half + 1) * NR] = arr
    out_full += np.asarray(bo, np.float32)
    return out_full


def run(inputs, trace=False, dt_name=None, add_w=None, **spmd_kwargs):
    """Run on hardware; returns (output, BassKernelResults)."""
    key = (dt_name or DT, add_w or ADD_W)
    if key not in _CACHE:
        _CACHE[key] = _build(dt_name=dt_name, add_w=add_w)
    nc = _CACHE[key]
    in_maps = _host_prep(
        np.asarray(inputs["input1"], np.float32),
        np.asarray(inputs["input2"], np.float32),
        np.asarray(inputs["W1"], np.float32),
        np.asarray(inputs["b1"], np.float32),
        np.asarray(inputs["W2"], np.float32),
        np.asarray(inputs["Wo"], np.float32),
        dt_name=dt_name,
    )
    res = run_bass_kernel_spmd(
        nc, in_maps, list(range(NCORES)), trace=trace, **spmd_kwargs
    )
    out = _host_post(res.results, np.asarray(inputs["bo"], np.float32))
    return out, res


def kernel(**inputs) -> np.ndarray:
    out, _ = run(inputs, trace=False)
    return out


if __name__ == "__main__":
    rng = np.random.default_rng(0)
    ins = {
        "input1": rng.standard_normal((B, N1, D), dtype=np.float32),
        "input2": rng.standard_normal((B, N2, D), dtype=np.float32),
        "W1": rng.standard_normal((HID, D), dtype=np.float32) * 0.036,
        "b1": rng.standard_normal((HID,), dtype=np.float32) * 0.036,
        "W2": rng.standard_normal((HID, D), dtype=np.float32) * 0.036,
        "Wo": rng.standard_normal((OUT, HID), dtype=np.float32) * 0.031,
        "bo": rng.standard_normal((OUT,), dtype=np.float32) * 0.031,
    }
    out = kernel(**ins)
    print("kernel out", out.shape, out.dtype)
